# revision 1
# baseline (speedup 1.0000x reference)
"""HAN (hierarchical attention network) Bass kernel for TRN2, 8-core SPMD.

Sharding: data-parallel over sentences for the word-level bi-GRU (12
sentences/core, fwd+bwd packed into one 24-lane batch padded to 32), one
AllGather of the 96 sentence vectors, then the sentence-level bi-GRU +
attention replicated on every core.

Layouts:
  - All GRU weight matrices are passed pre-transposed ([in, 3H]) with the
    3H columns permuted into 4 groups of [r256|z256|n256] so each PSUM
    group tile covers an aligned 256-slice of r/z/n.
  - Recurrent state h is kept two ways: batch-major [B,1024] f32 (gate
    math) and feature-major hT [128,8,B] bf16 (matmul stationary operand),
    rebuilt each step via 8 xbar DMA transposes.
  - Per-(step,lane) input projections xW live in DRAM [T*SPC, 3072] bf16
    (time-major), staged into SBUF per step, two steps in flight.
"""

import numpy as np

import concourse.bass as bass
import concourse.mybir as mybir
from concourse.tile import TileContext

FP32 = mybir.dt.float32
BF16 = mybir.dt.bfloat16
AF = mybir.ActivationFunctionType
OP = mybir.AluOpType

H = 1024
H3 = 3072
E = 1024
V = 50000
N_CORES = 8
BL = 12             # bwd lane offset; B lanes: 0:nf fwd, BL:BL+nf bwd
B = 32              # padded batch lanes
NG = 4              # rzn gate groups
GC = H3 // NG       # 768 cols per group (r256|z256|n256)
KH = H // 128       # k-chunks over H = 8


def gate_perm():
    """Column permutation of the 3H gate dim: 4 groups of [r256|z256|n256]."""
    p = []
    for g in range(NG):
        for blk in range(3):
            base = blk * H + g * 256
            p.extend(range(base, base + 256))
    return np.array(p, dtype=np.int64)


def emit_bcast128(nc, pool, psum_pool, src_sb, W, tag):
    """Replicate src_sb [1, W] f32 across partitions -> [128, W] f32 tile."""
    ones = pool.tile([1, 128], FP32, tag=f"{tag}_ones")
    nc.vector.memset(ones[:], 1.0)
    out = pool.tile([128, W], FP32, tag=f"{tag}_b128")
    for j in range(0, W, 512):
        w = min(512, W - j)
        ps = psum_pool.tile([128, 512], FP32, tag="bc_ps", name="bc_ps")
        nc.tensor.matmul(ps[:, :w], ones[:], src_sb[0:1, j:j + w],
                         start=True, stop=True)
        nc.vector.tensor_copy(out=out[:, j:j + w], in_=ps[:, :w])
    return out


def emit_load_bf16(nc, tmp_pool, dst, src_dram_rows, cols):
    """HWDGE f32 load + DVE cast (gpsimd cast-DMAs are ~0.3 GB/s here)."""
    tmp = tmp_pool.tile([128, cols], FP32, tag="ldtmp", name="ldtmp")
    nc.sync.dma_start(out=tmp[:, :cols], in_=src_dram_rows)
    nc.vector.tensor_copy(out=dst, in_=tmp[:, :cols])


def emit_projection(nc, pool, psum_pool, *, kc, m_tiles, lhsT_tiles,
                    w_sb, w_dram, bias_sb, out_dram):
    """out_dram[rows] = x @ W.T + bias (bf16), cols already in perm order.

    kc: 128-contraction chunks; m_tiles[i]: valid rows of tile i;
    lhsT_tiles[i]: sbuf AP [128, kc, rows_i] feature-major input chunk;
    weights: either resident w_sb [128, kc, 3072] bf16, or streamed per-k
    from w_dram [kc*128, 3072] f32; bias_sb: [1, 3072] f32.
    Loop order m -> k -> n6 with 6 live psum banks per m-tile.
    """
    r0 = 0
    for mi, mrows in enumerate(m_tiles):
        xw_tile = pool.tile([128, H3], BF16, tag="proj_xw")
        pss = [psum_pool.tile([128, 512], FP32, tag=f"proj_ps{j}",
                              name=f"proj_ps{j}") for j in range(6)]
        for k in range(kc):
            if w_sb is not None:
                wk = w_sb[:, k, :]
            else:
                wkt = pool.tile([128, H3], BF16, tag="proj_wk")
                emit_load_bf16(nc, pool, wkt[:, :], w_dram[k * 128:(k + 1) * 128, :], H3)
                wk = wkt[:, :]
            for n6 in range(6):
                nc.tensor.matmul(
                    pss[n6][:mrows, :],
                    lhsT_tiles[mi][:, k, :mrows],
                    wk[:, n6 * 512:(n6 + 1) * 512],
                    start=(k == 0), stop=(k == kc - 1),
                )
        for n6 in range(6):
            nc.vector.tensor_tensor(
                out=xw_tile[:mrows, n6 * 512:(n6 + 1) * 512],
                in0=pss[n6][:mrows, :],
                in1=bias_sb[:mrows, n6 * 512:(n6 + 1) * 512],
                op=OP.add,
            )
        nc.sync.dma_start(out=out_dram[r0:r0 + mrows, :], in_=xw_tile[:mrows, :])
        r0 += mrows


def emit_recurrence(nc, pool, wkpool, psum_pool, tpsum_pool, *, T, nf, x_d,
                    hidf_d, hidb_d, whh_sb, bhn_sb):
    """Bidirectional GRU, T steps, fwd lanes [0:nf], bwd lanes [BL:BL+nf].

    x_d: DRAM [T*nf, H3] bf16 time-major (perm'd cols).
    hidf_d/hidb_d: DRAM [T*nf, H] bf16, time-aligned (bwd stored at its
    logical time index). whh_sb: [128, KH, H3] bf16. bhn_sb: [1, H] f32
    n-part of b_hh (plain order) or None.
    """
    from concourse.masks import make_identity
    ident = pool.tile([B, B], FP32, tag="rc_ident")
    make_identity(nc, ident[:])
    stage = [pool.tile([B, H3], BF16, tag=f"rc_stage{i}", name=f"rc_stage{i}") for i in range(3)]
    hT = [pool.tile([128, KH, B], BF16, tag=f"rc_hT{i}", name=f"rc_hT{i}") for i in range(2)]
    h = pool.tile([B, H], FP32, tag="rc_h")
    hnb = pool.tile([B, H], BF16, tag="rc_hnb")
    for tl in stage + hT + [h, hnb]:
        nc.vector.memset(tl[:], 0.0)

    nb = min(BL + nf, B)  # active lane span
    for t in range(T):
        st = stage[t % 3]
        hT_cur, hT_nxt = hT[t % 2], hT[(t + 1) % 2]
        nc.sync.dma_start(out=st[0:nf, :], in_=x_d[t * nf:(t + 1) * nf, :])
        nc.sync.dma_start(out=st[BL:BL + nf, :],
                          in_=x_d[(T - 1 - t) * nf:(T - t) * nf, :])
        for g in range(NG):
            ps = psum_pool.tile([B, GC], FP32, tag="rc_ps")
            for k in range(KH):
                nc.tensor.matmul(ps[:, 0:512], hT_cur[:, k, :],
                                 whh_sb[:, k, g * GC:g * GC + 512],
                                 start=(k == 0), stop=(k == KH - 1))
                nc.tensor.matmul(ps[:, 512:768], hT_cur[:, k, :],
                                 whh_sb[:, k, g * GC + 512:(g + 1) * GC],
                                 start=(k == 0), stop=(k == KH - 1))
            hs = slice(g * 256, (g + 1) * 256)
            rz = wkpool.tile([B, 512], FP32, tag="rc_rz")
            sc1 = wkpool.tile([B, 256], FP32, tag="rc_sc1")
            # r,z = sigmoid(xw + hw)
            nc.vector.tensor_tensor(out=rz[:nb, :], in0=ps[:nb, 0:512],
                                    in1=st[:nb, g * GC:g * GC + 512], op=OP.add)
            nc.scalar.activation(rz[:nb, :], rz[:nb, :], AF.Sigmoid)
            # n = tanh(xn + r * (hn [+ bhn]))
            if bhn_sb is not None:
                nc.vector.tensor_tensor(
                    out=ps[:nb, 512:768], in0=ps[:nb, 512:768],
                    in1=bhn_sb[:nb, hs], op=OP.add)
            nc.vector.tensor_tensor(out=sc1[:nb, :], in0=rz[:nb, 0:256],
                                    in1=ps[:nb, 512:768], op=OP.mult)
            nc.vector.tensor_tensor(
                out=sc1[:nb, :], in0=sc1[:nb, :],
                in1=st[:nb, g * GC + 512:(g + 1) * GC], op=OP.add)
            nc.scalar.activation(sc1[:nb, :], sc1[:nb, :], AF.Tanh)
            # h' = n + z*(h-n)
            nc.vector.tensor_tensor(out=h[:nb, hs], in0=h[:nb, hs],
                                    in1=sc1[:nb, :], op=OP.subtract)
            nc.vector.tensor_tensor(out=h[:nb, hs], in0=h[:nb, hs],
                                    in1=rz[:nb, 256:512], op=OP.mult)
            nc.vector.tensor_tensor(out=h[:nb, hs], in0=h[:nb, hs],
                                    in1=sc1[:nb, :], op=OP.add)
            nc.scalar.copy(out=hnb[:nb, hs], in_=h[:nb, hs])
        nc.scalar.dma_start(out=hidf_d[t * nf:(t + 1) * nf, :], in_=hnb[0:nf, :])
        nc.scalar.dma_start(out=hidb_d[(T - 1 - t) * nf:(T - t) * nf, :],
                          in_=hnb[BL:BL + nf, :])
        for k in range(KH):
            tp = tpsum_pool.tile([128, B], FP32, tag="rc_tp")
            nc.tensor.transpose(tp[:], h[:, k * 128:(k + 1) * 128], ident[:])
            nc.scalar.copy(out=hT_nxt[:, k, :], in_=tp[:])


def emit_attention(nc, pool, psum_pool, *, T, nf, hidf_d, hidb_d,
                   wf_sb, wb_sb, bias_sb, out_dram):
    """scores = exp(bi . wctx + b); out[s] = sum_t scores[s,t] * bi[s,t].

    hid*_d: DRAM [T*nf, H] bf16 time-major. out_dram: [nf, 2H] f32.
    """
    hf = pool.tile([T, nf * H], BF16, tag="at_hf")
    hb = pool.tile([T, nf * H], BF16, tag="at_hb")
    nc.sync.dma_start(out=hf[:], in_=hidf_d[:, :].rearrange(
        "(t s) h -> t (s h)", t=T))
    nc.sync.dma_start(out=hb[:], in_=hidb_d[:, :].rearrange(
        "(t s) h -> t (s h)", t=T))
    scr = pool.tile([T, H], FP32, tag="at_scr")
    sco = pool.tile([T, nf], FP32, tag="at_sco")
    scob = pool.tile([T, nf], BF16, tag="at_scob")
    sco2 = pool.tile([T, nf], FP32, tag="at_sco2")
    for s in range(nf):
        nc.vector.tensor_tensor(out=scr[:], in0=hf[:, s * H:(s + 1) * H],
                                in1=wf_sb[:T, :], op=OP.mult)
        nc.vector.reduce_sum(out=sco[:, s:s + 1], in_=scr[:],
                             axis=mybir.AxisListType.X)
        nc.vector.tensor_tensor(out=scr[:], in0=hb[:, s * H:(s + 1) * H],
                                in1=wb_sb[:T, :], op=OP.mult)
        nc.vector.reduce_sum(out=sco2[:, s:s + 1], in_=scr[:],
                             axis=mybir.AxisListType.X)
    nc.vector.tensor_tensor(out=sco[:], in0=sco[:], in1=sco2[:], op=OP.add)
    nc.scalar.activation(sco[:], sco[:], AF.Exp,
                         bias=bias_sb[:T, 0:1])
    nc.vector.tensor_copy(out=scob[:], in_=sco[:])
    for s in range(nf):
        ps = psum_pool.tile([1, 2 * H], FP32, tag="at_ps")
        for half in range(2):
            src = hf if half == 0 else hb
            for j in range(2):
                nc.tensor.matmul(
                    ps[:, half * H + j * 512:half * H + (j + 1) * 512],
                    scob[:, s:s + 1],
                    src[:, s * H + j * 512:s * H + (j + 1) * 512],
                    start=True, stop=True)
        sv = pool.tile([1, 2 * H], FP32, tag="at_sv")
        nc.scalar.copy(out=sv[:], in_=ps[:])
        nc.sync.dma_start(out=out_dram[s:s + 1, :], in_=sv[:])


def emit_transposes(nc, pool, src_sb, kc, rows, tag):
    """src_sb [rows, kc*128] bf16 -> [128, kc, rows] bf16 feature-major."""
    out = pool.tile([128, kc, rows], BF16, tag=tag)
    for k in range(kc):
        nc.sync.dma_start_transpose(out[:, k, :],
                                    src_sb[:, k * 128:(k + 1) * 128])
    return out


def build(T=96, SPC=12, debug=False):
    S = SPC * N_CORES
    NTOK = T * SPC
    NTC = (NTOK + 127) // 128  # token chunks
    nc = bass.Bass("TRN2", num_devices=N_CORES)

    toks = nc.dram_tensor("toks", [NTC * 128], mybir.dt.int32, kind="ExternalInput")
    emb = nc.dram_tensor("emb", [V, E], FP32, kind="ExternalInput")
    wihT = nc.dram_tensor("wihT", [E, H3], FP32, kind="ExternalInput")
    whhT = nc.dram_tensor("whhT", [H, H3], FP32, kind="ExternalInput")
    wbx = nc.dram_tensor("wbx", [H3], FP32, kind="ExternalInput")   # b_ih+b_hh_rz, perm'd
    wbhn = nc.dram_tensor("wbhn", [H], FP32, kind="ExternalInput")  # b_hh n-part, plain
    sihT = nc.dram_tensor("sihT", [2 * H, H3], FP32, kind="ExternalInput")
    shhT = nc.dram_tensor("shhT", [H, H3], FP32, kind="ExternalInput")
    sbx = nc.dram_tensor("sbx", [H3], FP32, kind="ExternalInput")
    sbhn = nc.dram_tensor("sbhn", [H], FP32, kind="ExternalInput")
    wctx = nc.dram_tensor("wctx", [2 * H], FP32, kind="ExternalInput")
    wctxb = nc.dram_tensor("wctxb", [1], FP32, kind="ExternalInput")
    sctx = nc.dram_tensor("sctx", [2 * H], FP32, kind="ExternalInput")
    sctxb = nc.dram_tensor("sctxb", [1], FP32, kind="ExternalInput")

    kind_dbg = "ExternalOutput" if debug else "Internal"
    xw_d = nc.dram_tensor("xw_d", [NTOK, H3], BF16, kind=kind_dbg)
    hidf_d = nc.dram_tensor("hidf_d", [NTOK, H], BF16, kind=kind_dbg)
    hidb_d = nc.dram_tensor("hidb_d", [NTOK, H], BF16, kind=kind_dbg)
    xs_d = nc.dram_tensor("xs_d", [S, H3], BF16, kind=kind_dbg)
    hsf_d = nc.dram_tensor("hsf_d", [S, H], BF16, kind=kind_dbg)
    hsb_d = nc.dram_tensor("hsb_d", [S, H], BF16, kind=kind_dbg)
    cc_in = nc.dram_tensor("cc_in", [SPC, 2 * H], FP32, kind="Internal")
    cc_out = nc.dram_tensor("cc_out", [S, 2 * H], FP32, kind="Internal",
                            addr_space="Shared")
    out = nc.dram_tensor("out", [1, 2 * H], FP32, kind="ExternalOutput")

    with TileContext(nc) as tc:
        # ---- word phase ----
        with tc.tile_pool(name="wc", bufs=1) as wcpool:
            with tc.tile_pool(name="wcp", bufs=2, space="PSUM") as wcps:
                bx1 = wcpool.tile([1, H3], FP32, tag="bx1")
                nc.sync.dma_start(out=bx1[:], in_=wbx[None, :])
                bx_sb = emit_bcast128(nc, wcpool, wcps, bx1, H3, "bx")
                bhn1 = wcpool.tile([1, H], FP32, tag="bhn1")
                nc.sync.dma_start(out=bhn1[:], in_=wbhn[None, :])
                bhn_sb = emit_bcast128(nc, wcpool, wcps, bhn1, H, "bhn")

            with tc.tile_pool(name="wrw", bufs=1) as wrpool:
                whh_sb = wrpool.tile([128, KH, H3], BF16, tag="w_hh")
                with tc.tile_pool(name="wldt", bufs=2) as wldt:
                    for k in range(KH):
                        emit_load_bf16(nc, wldt, whh_sb[:, k, :],
                                       whhT[k * 128:(k + 1) * 128, :], H3)

                with tc.tile_pool(name="pj", bufs=1) as ppool, \
                     tc.tile_pool(name="pjw", bufs=2) as pwork, \
                     tc.tile_pool(name="pjp", bufs=1, space="PSUM") as pps:
                    wih_sb = ppool.tile([128, KH, H3], BF16, tag="w_ih")
                    for k in range(KH):
                        emit_load_bf16(nc, pwork, wih_sb[:, k, :],
                                       wihT[k * 128:(k + 1) * 128, :], H3)
                    tok_sb = ppool.tile([128, NTC], mybir.dt.int32, tag="tok")
                    for c in range(NTC):
                        nc.sync.dma_start(out=tok_sb[:, c:c + 1],
                                          in_=toks[c * 128:(c + 1) * 128][:, None])
                    lhsT_tiles = []
                    for c in range(NTC):
                        et = pwork.tile([128, E], FP32, tag="emb_f32")
                        nc.gpsimd.indirect_dma_start(
                            out=et[:], out_offset=None, in_=emb[:],
                            in_offset=bass.IndirectOffsetOnAxis(
                                ap=tok_sb[:, c:c + 1], axis=0))
                        eb = pwork.tile([128, E], BF16, tag="emb_bf")
                        nc.vector.tensor_copy(out=eb[:], in_=et[:])
                        lhsT_tiles.append(
                            emit_transposes(nc, ppool, eb, KH, 128, f"embT{c}"))
                    mrows = [128] * (NTOK // 128)
                    if NTOK % 128:
                        mrows.append(NTOK % 128)
                    emit_projection(nc, pwork, pps, kc=KH, m_tiles=mrows,
                                    lhsT_tiles=lhsT_tiles, w_sb=wih_sb,
                                    w_dram=None, bias_sb=bx_sb, out_dram=xw_d)

                with tc.tile_pool(name="rc", bufs=1) as rpool, \
                     tc.tile_pool(name="rcw", bufs=8) as rwork, \
                     tc.tile_pool(name="rcp", bufs=3, space="PSUM") as rps, \
                     tc.tile_pool(name="rct", bufs=2, space="PSUM") as rtps:
                    emit_recurrence(nc, rpool, rwork, rps, rtps, T=T, nf=SPC,
                                    x_d=xw_d, hidf_d=hidf_d, hidb_d=hidb_d,
                                    whh_sb=whh_sb, bhn_sb=bhn_sb)

            with tc.tile_pool(name="at", bufs=1) as apool, \
                 tc.tile_pool(name="atp", bufs=1, space="PSUM") as aps:
                wcf1 = apool.tile([1, H], FP32, tag="wcf1")
                wcb1 = apool.tile([1, H], FP32, tag="wcb1")
                nc.sync.dma_start(out=wcf1[:], in_=wctx[None, 0:H])
                nc.sync.dma_start(out=wcb1[:], in_=wctx[None, H:2 * H])
                wcbias1 = apool.tile([1, 1], FP32, tag="wcbias1")
                nc.sync.dma_start(out=wcbias1[:], in_=wctxb[None, :])
                wcf_sb = emit_bcast128(nc, apool, aps, wcf1, H, "wcf")
                wcb_sb = emit_bcast128(nc, apool, aps, wcb1, H, "wcb")
                wcbias_sb = emit_bcast128(nc, apool, aps, wcbias1, 1, "wcbias")
                emit_attention(nc, apool, aps, T=T, nf=SPC, hidf_d=hidf_d,
                               hidb_d=hidb_d, wf_sb=wcf_sb, wb_sb=wcb_sb,
                               bias_sb=wcbias_sb, out_dram=cc_in)

        nc.gpsimd.collective_compute(
            "AllGather", OP.bypass,
            ins=[cc_in[:, :]], outs=[cc_out[:, :]],
            replica_groups=[list(range(N_CORES))])

        # ---- sentence phase ----
        with tc.tile_pool(name="sc", bufs=1) as scpool:
            with tc.tile_pool(name="scps", bufs=2, space="PSUM") as scps:
                sbx1 = scpool.tile([1, H3], FP32, tag="sbx1")
                nc.sync.dma_start(out=sbx1[:], in_=sbx[None, :])
                sbx_sb = emit_bcast128(nc, scpool, scps, sbx1, H3, "sbx")
                sbhn1 = scpool.tile([1, H], FP32, tag="sbhn1")
                nc.sync.dma_start(out=sbhn1[:], in_=sbhn[None, :])
                sbhn_sb = emit_bcast128(nc, scpool, scps, sbhn1, H, "sbhn")

            with tc.tile_pool(name="srw", bufs=1) as srpool:
                shh_sb = srpool.tile([128, KH, H3], BF16, tag="s_hh")
                with tc.tile_pool(name="sldt", bufs=2) as sldt:
                    for k in range(KH):
                        emit_load_bf16(nc, sldt, shh_sb[:, k, :],
                                       shhT[k * 128:(k + 1) * 128, :], H3)

                with tc.tile_pool(name="sj", bufs=1) as sppool, \
                     tc.tile_pool(name="sjw", bufs=2) as spwork, \
                     tc.tile_pool(name="sjp", bufs=1, space="PSUM") as spps:
                    svb = sppool.tile([S, 2 * H], BF16, tag="svb")
                    svbt = spwork.tile([S, 2 * H], FP32, tag="svbt")
                    nc.sync.dma_start(out=svbt[:], in_=cc_out[:, :])
                    nc.vector.tensor_copy(out=svb[:], in_=svbt[:])
                    svT = emit_transposes(nc, sppool, svb, 2 * KH, S, "svT")
                    emit_projection(nc, spwork, spps, kc=2 * KH, m_tiles=[S],
                                    lhsT_tiles=[svT], w_sb=None, w_dram=sihT,
                                    bias_sb=sbx_sb, out_dram=xs_d)

                with tc.tile_pool(name="sr", bufs=1) as s_rpool, \
                     tc.tile_pool(name="srwk", bufs=8) as s_rwork, \
                     tc.tile_pool(name="srp", bufs=3, space="PSUM") as s_rps, \
                     tc.tile_pool(name="srt", bufs=2, space="PSUM") as s_rtps:
                    emit_recurrence(nc, s_rpool, s_rwork, s_rps, s_rtps, T=S, nf=1,
                                    x_d=xs_d, hidf_d=hsf_d, hidb_d=hsb_d,
                                    whh_sb=shh_sb, bhn_sb=sbhn_sb)

            with tc.tile_pool(name="sat", bufs=1) as sapool, \
                 tc.tile_pool(name="satp", bufs=1, space="PSUM") as saps:
                scf1 = sapool.tile([1, H], FP32, tag="scf1")
                scb1 = sapool.tile([1, H], FP32, tag="scb1")
                nc.sync.dma_start(out=scf1[:], in_=sctx[None, 0:H])
                nc.sync.dma_start(out=scb1[:], in_=sctx[None, H:2 * H])
                scbias1 = sapool.tile([1, 1], FP32, tag="scbias1")
                nc.sync.dma_start(out=scbias1[:], in_=sctxb[None, :])
                scf_sb = emit_bcast128(nc, sapool, saps, scf1, H, "scf")
                scb_sb = emit_bcast128(nc, sapool, saps, scb1, H, "scb")
                scbias_sb = emit_bcast128(nc, sapool, saps, scbias1, 1, "scbias")
                emit_attention(nc, sapool, saps, T=S, nf=1, hidf_d=hsf_d,
                               hidb_d=hsb_d, wf_sb=scf_sb, wb_sb=scb_sb,
                               bias_sb=scbias_sb, out_dram=out)

    return nc


def host_inputs(inputs, core, T=96, SPC=12):
    """Build the per-core in_map from the full problem inputs."""
    perm = gate_perm()
    NTOK = T * SPC
    NTC = (NTOK + 127) // 128
    tokens = np.asarray(inputs["tokens"])
    bih = np.asarray(inputs["w_bih"], np.float32)
    bhh = np.asarray(inputs["w_bhh"], np.float32)
    sbih = np.asarray(inputs["s_bih"], np.float32)
    sbhh = np.asarray(inputs["s_bhh"], np.float32)
    bx = bih.copy()
    bx[:2 * H] += bhh[:2 * H]
    sbx = sbih.copy()
    sbx[:2 * H] += sbhh[:2 * H]
    tk = tokens[core * SPC:(core + 1) * SPC, :T].T.reshape(-1).astype(np.int32)
    tk = np.concatenate([tk, np.zeros(NTC * 128 - NTOK, np.int32)])
    return {
        "toks": np.ascontiguousarray(tk),
        "emb": np.asarray(inputs["embedding"], np.float32),
        "wihT": np.ascontiguousarray(
            np.asarray(inputs["w_Wih"], np.float32).T[:, perm]),
        "whhT": np.ascontiguousarray(
            np.asarray(inputs["w_Whh"], np.float32).T[:, perm]),
        "wbx": np.ascontiguousarray(bx[perm]),
        "wbhn": np.ascontiguousarray(bhh[2 * H:]),
        "sihT": np.ascontiguousarray(
            np.asarray(inputs["s_Wih"], np.float32).T[:, perm]),
        "shhT": np.ascontiguousarray(
            np.asarray(inputs["s_Whh"], np.float32).T[:, perm]),
        "sbx": np.ascontiguousarray(sbx[perm]),
        "sbhn": np.ascontiguousarray(sbhh[2 * H:]),
        "wctx": np.asarray(inputs["wctx_w"], np.float32),
        "wctxb": np.asarray(inputs["wctx_b"], np.float32),
        "sctx": np.asarray(inputs["sctx_w"], np.float32),
        "sctxb": np.asarray(inputs["sctx_b"], np.float32),
    }


# ----- walrus sync-wait legalization (inlined) -----
import bass_rust
import concourse.mybir as mybir

MAX_WAITS = 1


def _expand_range_clear(ins):
    """EVENT_SEMAPHORE_RANGE_CLEAR InstISAs (opcode 176) trip this walrus
    ("ISA wrong length"). Replace each with per-semaphore sem-wr-imm 0
    EventSemaphore ops so re-execution of the loaded NEFF starts from
    clean semaphores."""
    import re

    m = re.search(r"range_first=(\d+) range_last=(\d+)", str(ins))
    assert m, f"cannot parse range clear: {ins}"
    lo, hi = int(m.group(1)), int(m.group(2))
    out = []
    for sem in range(lo, hi + 1):
        si = bass_rust.SyncInfo(
            on_wait=list(ins.sync_info.on_wait) if (
                ins.sync_info and sem == lo) else [],
            on_update=[bass_rust.SyncUpdate(
                sync_type="semaphore", id=sem, ant_name=f"semclr{sem}",
                update_mode="sem-wr-imm", update_value=0)],
        )
        out.append(mybir.InstEventSemaphore(
            name=f"{ins.name}-clr{sem}", engine=ins.engine, ins=[], outs=[],
            sync_info=si))
    return out


def split_waits(nc, max_waits: int = MAX_WAITS) -> int:
    n_new = 0
    for fn in nc.m.functions:
        for blk in fn.blocks:
            expanded = []
            for ins in blk.instructions:
                if (type(ins).__name__ == "InstISA"
                        and getattr(ins, "isa_opcode", None) == 176):
                    expanded.extend(_expand_range_clear(ins))
                else:
                    expanded.append(ins)
            blk.instructions[:] = expanded
            newlist = []
            for ins in blk.instructions:
                si = getattr(ins, "sync_info", None)
                ow = list(si.on_wait) if si and si.on_wait else []
                if len(ow) > max_waits:
                    extra = ow[max_waits:]
                    si.on_wait = ow[:max_waits]
                    for j in range(0, len(extra), max_waits):
                        nsi = bass_rust.SyncInfo(
                            on_wait=extra[j : j + max_waits], on_update=[]
                        )
                        nop = mybir.InstNoOp(
                            name=f"I-waitsplit-{n_new}",
                            engine=ins.engine,
                            ins=[],
                            outs=[],
                            sync_info=nsi,
                        )
                        newlist.append(nop)
                        n_new += 1
                newlist.append(ins)
            blk.instructions[:] = newlist
    return n_new


# ---------------------------------------------------------------------------
# Harness entry point: kernel(**inputs) -> np.ndarray  (full [2048] output)
# ---------------------------------------------------------------------------
_CACHE = {}


def _get_nc():
    if "nc" not in _CACHE:
        nc = build(T=96, SPC=12)
        split_waits(nc)
        _CACHE["nc"] = nc
    return _CACHE["nc"]


def kernel(**inputs):
    from concourse.bass_utils import run_bass_kernel_spmd

    nc = _get_nc()
    in_maps = [host_inputs(inputs, c) for c in range(N_CORES)]
    res = run_bass_kernel_spmd(nc, in_maps, core_ids=list(range(N_CORES)))
    return np.asarray(res.results[0]["out"][0], np.float32)


def _make_callable(nc, in_maps):
    """bass2jax multi-core dispatch without output donation, so the jitted
    callable can be re-invoked on device-resident inputs for timing."""
    import jax
    from jax.sharding import Mesh, PartitionSpec, NamedSharding
    from jax.experimental.shard_map import shard_map
    from concourse import bass2jax

    bass2jax.install_neuronx_cc_hook()
    pname = nc.partition_id_tensor.name if nc.partition_id_tensor else None
    in_names, out_names, out_avals, zero_outs = [], [], [], []
    for alloc in nc.m.functions[0].allocations:
        if not isinstance(alloc, mybir.MemoryLocationSet):
            continue
        name = alloc.memorylocations[0].name
        if alloc.kind == "ExternalInput":
            if name != pname:
                in_names.append(name)
        elif alloc.kind == "ExternalOutput":
            out_names.append(name)
            shape = tuple(alloc.tensor_shape)
            dtype = mybir.dt.np(alloc.dtype)
            out_avals.append(jax.core.ShapedArray(shape, dtype))
            zero_outs.append(np.zeros(shape, dtype))
    n_params = len(in_names)
    all_in = list(in_names) + list(out_names) + ([pname] if pname else [])

    def _body(*args):
        operands = list(args)
        if pname is not None:
            operands.append(bass2jax.partition_id_tensor())
        return tuple(bass2jax._bass_exec_p.bind(
            *operands, out_avals=tuple(out_avals), in_names=tuple(all_in),
            out_names=tuple(out_names), lowering_input_output_aliases=(),
            sim_require_finite=False, sim_require_nnan=False, nc=nc))

    devices = jax.devices()[:N_CORES]
    mesh = Mesh(np.asarray(devices), ("core",))
    spec = NamedSharding(mesh, PartitionSpec("core"))
    nio = n_params + len(out_names)
    fn = jax.jit(shard_map(_body, mesh=mesh,
                           in_specs=(PartitionSpec("core"),) * nio,
                           out_specs=(PartitionSpec("core"),) * len(out_names),
                           check_rep=False), keep_unused=True)
    cat = [np.concatenate([np.asarray(in_maps[c][k]) for c in range(N_CORES)],
                          axis=0) for k in in_names]
    cat += [np.zeros((N_CORES * z.shape[0], *z.shape[1:]), z.dtype)
            for z in zero_outs]
    dev_args = [jax.device_put(a, spec) for a in cat]
    return fn, dev_args, out_names, out_avals


def _time_callable(fn, dev_args, n):
    import time as _time
    import jax
    jax.block_until_ready(fn(*dev_args))
    best = float("inf")
    for _ in range(n):
        t0 = _time.perf_counter()
        jax.block_until_ready(fn(*dev_args))
        best = min(best, _time.perf_counter() - t0)
    return best * 1e9


def benchmark(inputs, n=10):
    """Returns (output, est_hw_ns, raw_wall_ns, floor_wall_ns). The axon
    dispatch round-trip (~80 ms) dominates wall time, so the HW estimate is
    the warm-wall delta vs an empty kernel measured the same way."""
    import concourse.bass as bass
    from concourse.tile import TileContext

    nf = bass.Bass("TRN2", num_devices=N_CORES)
    xf = nf.dram_tensor("x", [1, 128], FP32, kind="ExternalInput")
    yf = nf.dram_tensor("y", [1, 128], FP32, kind="ExternalOutput")
    with TileContext(nf) as tcf:
        with tcf.tile_pool(name="p", bufs=1) as pf:
            tt = pf.tile([1, 128], FP32, name="tt")
            nf.sync.dma_start(out=tt[:], in_=xf[:])
            nf.sync.dma_start(out=yf[:], in_=tt[:])
    split_waits(nf)
    ffn, fargs, _, _ = _make_callable(
        nf, [{"x": np.zeros((1, 128), np.float32)}] * N_CORES)
    floor_ns = _time_callable(ffn, fargs, max(n, 20))

    nc = _get_nc()
    in_maps = [host_inputs(inputs, c) for c in range(N_CORES)]
    fn, dev_args, out_names, out_avals = _make_callable(nc, in_maps)
    wall_ns = _time_callable(fn, dev_args, n)
    outs = fn(*dev_args)
    i = out_names.index("out")
    res = np.asarray(outs[i]).reshape(N_CORES, *out_avals[i].shape)[0]
    return np.asarray(res[0], np.float32), wall_ns - floor_ns, wall_ns, floor_ns



# revision 14
# speedup vs baseline: 4.8677x; 4.8677x over previous
"""HAN (hierarchical attention network) Bass kernel for TRN2, 8-core SPMD.

v2: fp8-DoubleRow GRU recurrences + PSUM-injected gate math.

Sharding: data-parallel over sentences for the word-level bi-GRU (12
sentences/core, fwd+bwd packed into one 24-lane batch padded to 32), one
AllGather of the 96 sentence vectors, then the sentence-level bi-GRU +
attention replicated on every core.

Key layouts / tricks:
  - All GRU weights are host-cast: W_hh as fp8e4 (scaled by 64) packed for
    DoubleRow [kc, 128, 2, 3072]; W_ih as bf16 (scaled by 64) so the
    precomputed xW comes out 64x-scaled with no extra device ops.
  - Gate columns permuted into 2 groups of [r512|z512|n512] so each group's
    PSUM slab holds a contiguous rz block (one sigmoid) and n block.
  - xW and biases are injected INTO PSUM via identity/ones matmuls on the
    PE; sigmoid/tanh read PSUM directly on the Act engine with scale=1/64
    (descale fused into the activation).
  - h state is bf16 [B, H]; per step 8 PE transposes build hT and gpsimd
    copies cast psum->fp8 DoubleRow stationary [128, kc, 2, B].
"""

import numpy as np

import concourse.bass as bass
import concourse.mybir as mybir
from concourse.tile import TileContext

FP32 = mybir.dt.float32
BF16 = mybir.dt.bfloat16
FP8 = mybir.dt.float8e4
AF = mybir.ActivationFunctionType
OP = mybir.AluOpType
DR = mybir.MatmulPerfMode.DoubleRow

H = 1024
H3 = 3072
E = 1024
V = 50000
N_CORES = 8
BL = 12             # bwd lane offset; lanes: 0:nf fwd, BL:BL+nf bwd
B = 32              # padded batch lanes
NG = 4              # gate groups, each [r|z|n] of GH; group g == fp8 k-chunk g
GC = H3 // NG       # 1536 cols per group
GH = H // NG        # 512 h-dims per group
KH = H // 128       # bf16 contraction chunks = 8
KC = H // 256       # fp8 DoubleRow contraction chunks = 4
SCALE = 64.0


def gate_perm():
    """Column permutation of 3H: NG groups of [r|z|n] slices of GH each."""
    p = []
    for g in range(NG):
        for blk in range(3):
            base = blk * H + g * GH
            p.extend(range(base, base + GH))
    return np.array(p, dtype=np.int64)


def emit_projection(nc, pool, psum_pool, *, kc, m_tiles, lhsT_tiles,
                    w_sb, bias_sb, out_dram):
    """out_dram[rows] = 64*(x @ W.T + bias) in bf16, cols in perm order.

    kc: 128-contraction chunks; m_tiles[i]: valid rows of tile i;
    lhsT_tiles[i]: sbuf AP [128, kc, rows_i] bf16 feature-major input;
    w_sb: [128, kc, 3072] bf16 (64x-scaled); bias_sb: [1, 3072] bf16
    (64x-scaled), injected via a K=1 ones matmul.
    """
    ones1 = pool.tile([1, 128], BF16, tag="pj_ones")
    nc.vector.memset(ones1[:], 1.0)
    r0 = 0
    for mi, mrows in enumerate(m_tiles):
        xw_tile = pool.tile([128, H3], BF16, tag="proj_xw")
        pss = [psum_pool.tile([128, 512], FP32, tag=f"proj_ps{j}",
                              name=f"proj_ps{j}") for j in range(6)]
        for n6 in range(6):
            nc.tensor.matmul(pss[n6][:mrows, :], ones1[0:1, :mrows],
                             bias_sb[0:1, n6 * 512:(n6 + 1) * 512],
                             start=True, stop=False)
        for k in range(kc):
            for n6 in range(6):
                nc.tensor.matmul(
                    pss[n6][:mrows, :],
                    lhsT_tiles[mi][:, k, :mrows],
                    w_sb[:, k, n6 * 512:(n6 + 1) * 512],
                    start=False, stop=(k == kc - 1),
                )
        for n6 in range(6):
            nc.scalar.activation(xw_tile[:mrows, n6 * 512:(n6 + 1) * 512],
                                 pss[n6][:mrows, :], AF.Copy)
        nc.sync.dma_start(out=out_dram[r0:r0 + mrows, :], in_=xw_tile[:mrows, :])
        r0 += mrows


def emit_recurrence(nc, pool, wkpool, psum_pool, tpsum_pool, *, T, nf, x_d,
                    hidf_d, hidb_d, whh8_sb, nb_sb, dma_t_last=False):
    """Bidirectional GRU, T steps, fwd lanes [0:nf], bwd lanes [BL:BL+nf].

    x_d: DRAM [T*nf, H3] bf16 time-major, 64x-scaled, perm'd cols.
    hidf_d/hidb_d: DRAM [T*nf, H] bf16, time-aligned.
    whh8_sb: [128, KC, 2, H3] fp8 (64x-scaled, perm'd, DoubleRow packed).
    nb_sb: [1, H] bf16 = 64*b_hh_n in plain h order.

    Gate group g covers h-dims [256g:256g+256] == fp8 k-chunk g, so the next
    step's k=0 matmuls depend only on group 0's output. All matmuls for a
    step are emitted before any gate math, and the last group's hT rebuild
    goes through a DMA transpose so the in-order PE stream never stalls on
    the slowest gate-math chain.
    """
    from concourse.masks import make_identity
    identB = pool.tile([B, B], BF16, tag="rc_ident")
    make_identity(nc, identB[:])
    ones1 = pool.tile([1, B], BF16, tag="rc_ones")
    nc.vector.memset(ones1[:], 1.0)

    stage = [pool.tile([B, H3], BF16, tag=f"rc_stage{i}", name=f"rc_stage{i}")
             for i in range(4)]
    hT8 = [[pool.tile([128, 2, B], FP8, tag=f"rc_hT{i}_{c}",
                      name=f"rc_hT{i}_{c}") for c in range(KC)]
           for i in range(2)]
    h = pool.tile([B, H], BF16, tag="rc_h")
    for tl in stage + hT8[0] + hT8[1] + [h]:
        nc.vector.memset(tl[:], 0.0)

    gpsum = [psum_pool.tile([B, GC], FP32, tag=f"rc_ps{g}", name=f"rc_ps{g}")
             for g in range(2)]  # groups g and g+2 time-share a slab

    nb = min(BL + nf, B)  # active lane span
    for t in range(T):
        st = stage[t % 4]
        hT_cur, hT_nxt = hT8[t % 2], hT8[(t + 1) % 2]
        nc.sync.dma_start(out=st[0:nf, :], in_=x_d[t * nf:(t + 1) * nf, :])
        nc.sync.dma_start(out=st[BL:BL + nf, :],
                          in_=x_d[(T - 1 - t) * nf:(T - t) * nf, :])
        def emit_mm(g):
            ps = gpsum[g % 2]
            c0 = g * GC
            # start/stop are per 2KB psum zero-region: [0:512] is region 0,
            # [512:768] region 1; each region's first/last write carries them
            nc.tensor.matmul(ps[:, 0:512], identB[:], st[:, c0:c0 + 512],
                             start=True, stop=False)
            nc.tensor.matmul(ps[:, 512:768], ones1[:],
                             nb_sb[0:1, g * GH:(g + 1) * GH],
                             start=True, stop=False)
            for k in range(KC):
                for n3 in range(3):
                    nc.tensor.matmul(
                        ps[:, n3 * 256:(n3 + 1) * 256],
                        hT_cur[k][:, :, :],
                        whh8_sb[:, k, :, c0 + n3 * 256:c0 + (n3 + 1) * 256],
                        start=False, stop=(k == KC - 1 and n3 >= 1),
                        perf_mode=DR,
                    )

        def emit_math(g):
            ps = gpsum[g % 2]
            c0 = g * GC
            hs = slice(g * GH, (g + 1) * GH)
            rz = wkpool.tile([B, 512], BF16, tag="rc_rz")
            nt = wkpool.tile([B, GH], BF16, tag="rc_nt")
            hm = wkpool.tile([B, GH], BF16, tag="rc_hm")
            # r,z = sigmoid((xw + hw)/64)
            nc.scalar.activation(rz[:nb, :], ps[:nb, 0:512], AF.Sigmoid,
                                 scale=1.0 / SCALE)
            # n = tanh((r*(hn + bhn) + xn)/64)
            nc.vector.tensor_tensor(out=nt[:nb, :], in0=rz[:nb, 0:GH],
                                    in1=ps[:nb, 512:768], op=OP.mult)
            nc.vector.tensor_tensor(out=nt[:nb, :], in0=nt[:nb, :],
                                    in1=st[:nb, c0 + 512:c0 + 768], op=OP.add)
            nc.scalar.activation(nt[:nb, :], nt[:nb, :], AF.Tanh,
                                 scale=1.0 / SCALE)
            # h' = n + z*(h-n)
            nc.vector.tensor_tensor(out=hm[:nb, :], in0=h[:nb, hs],
                                    in1=nt[:nb, :], op=OP.subtract)
            nc.vector.tensor_tensor(out=hm[:nb, :], in0=hm[:nb, :],
                                    in1=rz[:nb, GH:512], op=OP.mult)
            nc.vector.tensor_tensor(out=h[:nb, hs], in0=hm[:nb, :],
                                    in1=nt[:nb, :], op=OP.add)
            # rebuild hT fp8 chunk g (2 sub-chunks of 128 h-dims)
            if dma_t_last and g == NG - 1:
                for s in range(2):
                    blk = g * 2 + s
                    tb = wkpool.tile([128, B], BF16, tag="rc_tdma")
                    nc.sync.dma_start_transpose(
                        tb[:], h[:, blk * 128:(blk + 1) * 128])
                    nc.gpsimd.tensor_copy(out=hT_nxt[g][:, s, :], in_=tb[:])
            else:
                tp = tpsum_pool.tile([128, 2, B], BF16, tag="rc_tp")
                for s in range(2):
                    blk = g * 2 + s
                    nc.tensor.transpose(tp[:, s, :],
                                        h[:, blk * 128:(blk + 1) * 128],
                                        identB[:])
                nc.vector.tensor_copy(out=hT_nxt[g][:, :, :], in_=tp[:])

        # software-pipelined: psum slab g%2 is reused by g+2, so interleave
        emit_mm(0)
        emit_mm(1)
        emit_math(0)
        emit_mm(2)
        emit_math(1)
        emit_mm(3)
        emit_math(2)
        emit_math(3)
        nc.scalar.dma_start(out=hidf_d[t * nf:(t + 1) * nf, :], in_=h[0:nf, :])
        nc.scalar.dma_start(out=hidb_d[(T - 1 - t) * nf:(T - t) * nf, :],
                            in_=h[BL:BL + nf, :])


def emit_attention(nc, pool, psum_pool, *, T, nf, hidf_d, hidb_d,
                   wf_sb, wb_sb, bias_sb, out_dram):
    """scores = exp(bi . wctx + b); out[s] = sum_t scores[s,t] * bi[s,t].

    hid*_d: DRAM [T*nf, H] bf16 time-major. out_dram: [nf, 2H] f32.
    """
    hf = pool.tile([T, nf * H], BF16, tag="at_hf")
    hb = pool.tile([T, nf * H], BF16, tag="at_hb")
    nc.sync.dma_start(out=hf[:], in_=hidf_d[:, :].rearrange(
        "(t s) h -> t (s h)", t=T))
    nc.sync.dma_start(out=hb[:], in_=hidb_d[:, :].rearrange(
        "(t s) h -> t (s h)", t=T))
    scr = pool.tile([T, H], BF16, tag="at_scr")
    sco = pool.tile([T, nf], FP32, tag="at_sco")
    scob = pool.tile([T, nf], BF16, tag="at_scob")
    sco2 = pool.tile([T, nf], FP32, tag="at_sco2")
    for s in range(nf):
        nc.vector.tensor_tensor(out=scr[:], in0=hf[:, s * H:(s + 1) * H],
                                in1=wf_sb[:T, :], op=OP.mult)
        nc.vector.reduce_sum(out=sco[:, s:s + 1], in_=scr[:],
                             axis=mybir.AxisListType.X)
        nc.vector.tensor_tensor(out=scr[:], in0=hb[:, s * H:(s + 1) * H],
                                in1=wb_sb[:T, :], op=OP.mult)
        nc.vector.reduce_sum(out=sco2[:, s:s + 1], in_=scr[:],
                             axis=mybir.AxisListType.X)
    nc.vector.tensor_tensor(out=sco[:], in0=sco[:], in1=sco2[:], op=OP.add)
    nc.scalar.activation(sco[:], sco[:], AF.Exp,
                         bias=bias_sb[:T, 0:1])
    nc.vector.tensor_copy(out=scob[:], in_=sco[:])
    for s in range(nf):
        ps = psum_pool.tile([1, 2 * H], FP32, tag="at_ps")
        for half in range(2):
            src = hf if half == 0 else hb
            for j in range(2):
                nc.tensor.matmul(
                    ps[:, half * H + j * 512:half * H + (j + 1) * 512],
                    scob[:, s:s + 1],
                    src[:, s * H + j * 512:s * H + (j + 1) * 512],
                    start=True, stop=True)
        sv = pool.tile([1, 2 * H], FP32, tag="at_sv")
        nc.scalar.copy(out=sv[:], in_=ps[:])
        nc.sync.dma_start(out=out_dram[s:s + 1, :], in_=sv[:])


def emit_transposes(nc, pool, src_sb, kc, rows, tag):
    """src_sb [rows, kc*128] bf16 -> [128, kc, rows] bf16 feature-major."""
    out = pool.tile([128, kc, rows], BF16, tag=tag)
    for k in range(kc):
        nc.sync.dma_start_transpose(out[:, k, :],
                                    src_sb[:, k * 128:(k + 1) * 128])
    return out


def emit_bcast128(nc, pool, psum_pool, src_sb, W, tag, dtype=FP32):
    """Replicate src_sb [1, W] f32 across partitions -> [128, W] tile."""
    ones = pool.tile([1, 128], FP32, tag=f"{tag}_ones")
    nc.vector.memset(ones[:], 1.0)
    out = pool.tile([128, W], dtype, tag=f"{tag}_b128")
    for j in range(0, W, 512):
        w = min(512, W - j)
        ps = psum_pool.tile([128, 512], FP32, tag="bc_ps", name="bc_ps")
        nc.tensor.matmul(ps[:, :w], ones[:], src_sb[0:1, j:j + w],
                         start=True, stop=True)
        nc.vector.tensor_copy(out=out[:, j:j + w], in_=ps[:, :w])
    return out


def build(T=96, SPC=12, debug=False):
    S = SPC * N_CORES
    NTOK = T * SPC
    NTC = (NTOK + 127) // 128  # token chunks
    nc = bass.Bass("TRN2", num_devices=N_CORES)

    toks = nc.dram_tensor("toks", [NTC * 128], mybir.dt.int32, kind="ExternalInput")
    emb = nc.dram_tensor("emb", [V, E], BF16, kind="ExternalInput")
    wihT = nc.dram_tensor("wihT", [E, H3], BF16, kind="ExternalInput")
    whh8 = nc.dram_tensor("whh8", [KC, 128, 2, H3], FP8, kind="ExternalInput")
    wbx = nc.dram_tensor("wbx", [1, H3], BF16, kind="ExternalInput")   # 64*(bih+bhh_rz), perm'd
    wbhn = nc.dram_tensor("wbhn", [1, H], BF16, kind="ExternalInput")  # 64*bhh_n, plain
    sihT = nc.dram_tensor("sihT", [2 * H, H3], BF16, kind="ExternalInput")
    shh8 = nc.dram_tensor("shh8", [KC, 128, 2, H3], FP8, kind="ExternalInput")
    sbx = nc.dram_tensor("sbx", [1, H3], BF16, kind="ExternalInput")
    sbhn = nc.dram_tensor("sbhn", [1, H], BF16, kind="ExternalInput")
    wctx = nc.dram_tensor("wctx", [2 * H], FP32, kind="ExternalInput")
    wctxb = nc.dram_tensor("wctxb", [1], FP32, kind="ExternalInput")
    sctx = nc.dram_tensor("sctx", [2 * H], FP32, kind="ExternalInput")
    sctxb = nc.dram_tensor("sctxb", [1], FP32, kind="ExternalInput")

    kind_dbg = "ExternalOutput" if debug else "Internal"
    xw_d = nc.dram_tensor("xw_d", [NTOK, H3], BF16, kind=kind_dbg)
    hidf_d = nc.dram_tensor("hidf_d", [NTOK, H], BF16, kind=kind_dbg)
    hidb_d = nc.dram_tensor("hidb_d", [NTOK, H], BF16, kind=kind_dbg)
    xs_d = nc.dram_tensor("xs_d", [S, H3], BF16, kind=kind_dbg)
    hsf_d = nc.dram_tensor("hsf_d", [S, H], BF16, kind=kind_dbg)
    hsb_d = nc.dram_tensor("hsb_d", [S, H], BF16, kind=kind_dbg)
    cc_in = nc.dram_tensor("cc_in", [SPC, 2 * H], FP32, kind="Internal")
    cc_out = nc.dram_tensor("cc_out", [S, 2 * H], FP32, kind="Internal",
                            addr_space="Shared")
    out = nc.dram_tensor("out", [1, 2 * H], FP32, kind="ExternalOutput")

    with TileContext(nc) as tc:
        # ---- word phase ----
        with tc.tile_pool(name="wrw", bufs=1) as wrpool:
            whh_sb = wrpool.tile([128, KC, 2, H3], FP8, tag="w_hh")
            nc.sync.dma_start(out=whh_sb[:], in_=whh8[:, :, :, :].rearrange(
                "c p s j -> p c s j"))
            wbx_sb = wrpool.tile([1, H3], BF16, tag="wbx")
            nc.sync.dma_start(out=wbx_sb[:], in_=wbx[:, :])
            wnb_sb = wrpool.tile([1, H], BF16, tag="wnb")
            nc.sync.dma_start(out=wnb_sb[:], in_=wbhn[:, :])

            with tc.tile_pool(name="pj", bufs=1) as ppool, \
                 tc.tile_pool(name="pjw", bufs=2) as pwork, \
                 tc.tile_pool(name="pjp", bufs=1, space="PSUM") as pps:
                wih_sb = ppool.tile([128, KH, H3], BF16, tag="w_ih")
                for k in range(KH):
                    nc.sync.dma_start(out=wih_sb[:, k, :],
                                      in_=wihT[k * 128:(k + 1) * 128, :])
                tok_sb = ppool.tile([128, NTC], mybir.dt.int32, tag="tok")
                for c in range(NTC):
                    nc.sync.dma_start(out=tok_sb[:, c:c + 1],
                                      in_=toks[c * 128:(c + 1) * 128][:, None])
                lhsT_tiles = []
                for c in range(NTC):
                    eb = pwork.tile([128, E], BF16, tag="emb_bf")
                    nc.gpsimd.indirect_dma_start(
                        out=eb[:], out_offset=None, in_=emb[:],
                        in_offset=bass.IndirectOffsetOnAxis(
                            ap=tok_sb[:, c:c + 1], axis=0))
                    lhsT_tiles.append(
                        emit_transposes(nc, ppool, eb, KH, 128, f"embT{c}"))
                mrows = [128] * (NTOK // 128)
                if NTOK % 128:
                    mrows.append(NTOK % 128)
                emit_projection(nc, pwork, pps, kc=KH, m_tiles=mrows,
                                lhsT_tiles=lhsT_tiles, w_sb=wih_sb,
                                bias_sb=wbx_sb, out_dram=xw_d)

            with tc.tile_pool(name="rc", bufs=1) as rpool, \
                 tc.tile_pool(name="rcw", bufs=8) as rwork, \
                 tc.tile_pool(name="rcp", bufs=1, space="PSUM") as rps, \
                 tc.tile_pool(name="rct", bufs=2, space="PSUM") as rtps:
                emit_recurrence(nc, rpool, rwork, rps, rtps, T=T, nf=SPC,
                                x_d=xw_d, hidf_d=hidf_d, hidb_d=hidb_d,
                                whh8_sb=whh_sb, nb_sb=wnb_sb)

        # sentence-phase weights: load while attention runs
        with tc.tile_pool(name="srw", bufs=1) as srpool:
            shh_sb = srpool.tile([128, KC, 2, H3], FP8, tag="s_hh")
            nc.sync.dma_start(out=shh_sb[:], in_=shh8[:, :, :, :].rearrange(
                "c p s j -> p c s j"))
            sbx_sb = srpool.tile([1, H3], BF16, tag="sbx")
            nc.sync.dma_start(out=sbx_sb[:], in_=sbx[:, :])
            snb_sb = srpool.tile([1, H], BF16, tag="snb")
            nc.sync.dma_start(out=snb_sb[:], in_=sbhn[:, :])
            sih_sb = srpool.tile([128, 2 * KH, H3], BF16, tag="s_ih")
            for k in range(2 * KH):
                nc.sync.dma_start(out=sih_sb[:, k, :],
                                  in_=sihT[k * 128:(k + 1) * 128, :])

            with tc.tile_pool(name="at", bufs=1) as apool, \
                 tc.tile_pool(name="atp", bufs=1, space="PSUM") as aps:
                wcf1 = apool.tile([1, H], FP32, tag="wcf1")
                wcb1 = apool.tile([1, H], FP32, tag="wcb1")
                nc.sync.dma_start(out=wcf1[:], in_=wctx[None, 0:H])
                nc.sync.dma_start(out=wcb1[:], in_=wctx[None, H:2 * H])
                wcbias1 = apool.tile([1, 1], FP32, tag="wcbias1")
                nc.sync.dma_start(out=wcbias1[:], in_=wctxb[None, :])
                wcf_sb = emit_bcast128(nc, apool, aps, wcf1, H, "wcf", dtype=BF16)
                wcb_sb = emit_bcast128(nc, apool, aps, wcb1, H, "wcb", dtype=BF16)
                wcbias_sb = emit_bcast128(nc, apool, aps, wcbias1, 1, "wcbias")
                emit_attention(nc, apool, aps, T=T, nf=SPC, hidf_d=hidf_d,
                               hidb_d=hidb_d, wf_sb=wcf_sb, wb_sb=wcb_sb,
                               bias_sb=wcbias_sb, out_dram=cc_in)

            nc.gpsimd.collective_compute(
                "AllGather", OP.bypass,
                ins=[cc_in[:, :]], outs=[cc_out[:, :]],
                replica_groups=[list(range(N_CORES))])

            # ---- sentence phase ----
            with tc.tile_pool(name="sj", bufs=1) as sppool, \
                 tc.tile_pool(name="sjw", bufs=2) as spwork, \
                 tc.tile_pool(name="sjp", bufs=1, space="PSUM") as spps:
                svb = sppool.tile([S, 2 * H], BF16, tag="svb")
                svbt = spwork.tile([S, 2 * H], FP32, tag="svbt")
                nc.sync.dma_start(out=svbt[:], in_=cc_out[:, :])
                nc.vector.tensor_copy(out=svb[:], in_=svbt[:])
                svT = emit_transposes(nc, sppool, svb, 2 * KH, S, "svT")
                emit_projection(nc, spwork, spps, kc=2 * KH, m_tiles=[S],
                                lhsT_tiles=[svT], w_sb=sih_sb,
                                bias_sb=sbx_sb, out_dram=xs_d)

            with tc.tile_pool(name="sr", bufs=1) as s_rpool, \
                 tc.tile_pool(name="srwk", bufs=8) as s_rwork, \
                 tc.tile_pool(name="srp", bufs=1, space="PSUM") as s_rps, \
                 tc.tile_pool(name="srt", bufs=2, space="PSUM") as s_rtps:
                emit_recurrence(nc, s_rpool, s_rwork, s_rps, s_rtps, T=S, nf=1,
                                x_d=xs_d, hidf_d=hsf_d, hidb_d=hsb_d,
                                whh8_sb=shh_sb, nb_sb=snb_sb)

            with tc.tile_pool(name="sat", bufs=1) as sapool, \
                 tc.tile_pool(name="satp", bufs=1, space="PSUM") as saps:
                scf1 = sapool.tile([1, H], FP32, tag="scf1")
                scb1 = sapool.tile([1, H], FP32, tag="scb1")
                nc.sync.dma_start(out=scf1[:], in_=sctx[None, 0:H])
                nc.sync.dma_start(out=scb1[:], in_=sctx[None, H:2 * H])
                scbias1 = sapool.tile([1, 1], FP32, tag="scbias1")
                nc.sync.dma_start(out=scbias1[:], in_=sctxb[None, :])
                scf_sb = emit_bcast128(nc, sapool, saps, scf1, H, "scf", dtype=BF16)
                scb_sb = emit_bcast128(nc, sapool, saps, scb1, H, "scb", dtype=BF16)
                scbias_sb = emit_bcast128(nc, sapool, saps, scbias1, 1, "scbias")
                emit_attention(nc, sapool, saps, T=S, nf=1, hidf_d=hsf_d,
                               hidb_d=hsb_d, wf_sb=scf_sb, wb_sb=scb_sb,
                               bias_sb=scbias_sb, out_dram=out)

    return nc


def pack_dr(W64):
    """[H, H3] f32 (64x, perm'd cols) -> DoubleRow fp8 [KC, 128, 2, H3]."""
    import ml_dtypes
    return np.ascontiguousarray(
        W64.reshape(KC, 2, 128, H3).transpose(0, 2, 1, 3)
    ).astype(mybir.dt.np(FP8))


def host_inputs(inputs, core, T=96, SPC=12):
    """Build the per-core in_map from the full problem inputs."""
    import ml_dtypes
    bfn = mybir.dt.np(BF16)
    perm = gate_perm()
    NTOK = T * SPC
    NTC = (NTOK + 127) // 128
    tokens = np.asarray(inputs["tokens"])
    bih = np.asarray(inputs["w_bih"], np.float32)
    bhh = np.asarray(inputs["w_bhh"], np.float32)
    sbih = np.asarray(inputs["s_bih"], np.float32)
    sbhh = np.asarray(inputs["s_bhh"], np.float32)
    bx = bih.copy()
    bx[:2 * H] += bhh[:2 * H]
    sbx = sbih.copy()
    sbx[:2 * H] += sbhh[:2 * H]
    tk = tokens[core * SPC:(core + 1) * SPC, :T].T.reshape(-1).astype(np.int32)
    tk = np.concatenate([tk, np.zeros(NTC * 128 - NTOK, np.int32)])
    wih64 = SCALE * np.asarray(inputs["w_Wih"], np.float32)
    whh64 = SCALE * np.asarray(inputs["w_Whh"], np.float32)
    sih64 = SCALE * np.asarray(inputs["s_Wih"], np.float32)
    shh64 = SCALE * np.asarray(inputs["s_Whh"], np.float32)
    return {
        "toks": np.ascontiguousarray(tk),
        "emb": np.asarray(inputs["embedding"], np.float32).astype(bfn),
        "wihT": np.ascontiguousarray(wih64.T[:, perm]).astype(bfn),
        "whh8": pack_dr(np.ascontiguousarray(whh64.T[:, perm])),
        "wbx": np.ascontiguousarray(SCALE * bx[perm])[None, :].astype(bfn),
        "wbhn": np.ascontiguousarray(SCALE * bhh[2 * H:])[None, :].astype(bfn),
        "sihT": np.ascontiguousarray(sih64.T[:, perm]).astype(bfn),
        "shh8": pack_dr(np.ascontiguousarray(shh64.T[:, perm])),
        "sbx": np.ascontiguousarray(SCALE * sbx[perm])[None, :].astype(bfn),
        "sbhn": np.ascontiguousarray(SCALE * sbhh[2 * H:])[None, :].astype(bfn),
        "wctx": np.asarray(inputs["wctx_w"], np.float32),
        "wctxb": np.asarray(inputs["wctx_b"], np.float32),
        "sctx": np.asarray(inputs["sctx_w"], np.float32),
        "sctxb": np.asarray(inputs["sctx_b"], np.float32),
    }


# ----- walrus sync-wait legalization (inlined) -----
import bass_rust

MAX_WAITS = 1


def _expand_range_clear(ins):
    """EVENT_SEMAPHORE_RANGE_CLEAR InstISAs (opcode 176) trip this walrus
    ("ISA wrong length"). Replace each with per-semaphore sem-wr-imm 0
    EventSemaphore ops so re-execution of the loaded NEFF starts from
    clean semaphores."""
    import re

    m = re.search(r"range_first=(\d+) range_last=(\d+)", str(ins))
    assert m, f"cannot parse range clear: {ins}"
    lo, hi = int(m.group(1)), int(m.group(2))
    out = []
    for sem in range(lo, hi + 1):
        si = bass_rust.SyncInfo(
            on_wait=list(ins.sync_info.on_wait) if (
                ins.sync_info and sem == lo) else [],
            on_update=[bass_rust.SyncUpdate(
                sync_type="semaphore", id=sem, ant_name=f"semclr{sem}",
                update_mode="sem-wr-imm", update_value=0)],
        )
        out.append(mybir.InstEventSemaphore(
            name=f"{ins.name}-clr{sem}", engine=ins.engine, ins=[], outs=[],
            sync_info=si))
    return out


def split_waits(nc, max_waits: int = MAX_WAITS) -> int:
    n_new = 0
    for fn in nc.m.functions:
        for blk in fn.blocks:
            expanded = []
            for ins in blk.instructions:
                if (type(ins).__name__ == "InstISA"
                        and getattr(ins, "isa_opcode", None) == 176):
                    expanded.extend(_expand_range_clear(ins))
                else:
                    expanded.append(ins)
            blk.instructions[:] = expanded
            newlist = []
            for ins in blk.instructions:
                si = getattr(ins, "sync_info", None)
                ow = list(si.on_wait) if si and si.on_wait else []
                if len(ow) > max_waits:
                    extra = ow[max_waits:]
                    si.on_wait = ow[:max_waits]
                    for j in range(0, len(extra), max_waits):
                        nsi = bass_rust.SyncInfo(
                            on_wait=extra[j : j + max_waits], on_update=[]
                        )
                        nop = mybir.InstNoOp(
                            name=f"I-waitsplit-{n_new}",
                            engine=ins.engine,
                            ins=[],
                            outs=[],
                            sync_info=nsi,
                        )
                        newlist.append(nop)
                        n_new += 1
                newlist.append(ins)
            blk.instructions[:] = newlist
    return n_new


# ---------------------------------------------------------------------------
# Harness entry point: kernel(**inputs) -> np.ndarray  (full [2048] output)
# ---------------------------------------------------------------------------
_CACHE = {}


def _get_nc():
    if "nc" not in _CACHE:
        nc = build(T=96, SPC=12)
        split_waits(nc)
        _CACHE["nc"] = nc
    return _CACHE["nc"]


def kernel(**inputs):
    from concourse.bass_utils import run_bass_kernel_spmd

    nc = _get_nc()
    in_maps = [host_inputs(inputs, c) for c in range(N_CORES)]
    res = run_bass_kernel_spmd(nc, in_maps, core_ids=list(range(N_CORES)))
    return np.asarray(res.results[0]["out"][0], np.float32)


def _make_callable(nc, in_maps):
    """bass2jax multi-core dispatch without output donation, so the jitted
    callable can be re-invoked on device-resident inputs for timing."""
    import jax
    from jax.sharding import Mesh, PartitionSpec, NamedSharding
    from jax.experimental.shard_map import shard_map
    from concourse import bass2jax

    bass2jax.install_neuronx_cc_hook()
    pname = nc.partition_id_tensor.name if nc.partition_id_tensor else None
    in_names, out_names, out_avals, zero_outs = [], [], [], []
    for alloc in nc.m.functions[0].allocations:
        if not isinstance(alloc, mybir.MemoryLocationSet):
            continue
        name = alloc.memorylocations[0].name
        if alloc.kind == "ExternalInput":
            if name != pname:
                in_names.append(name)
        elif alloc.kind == "ExternalOutput":
            out_names.append(name)
            shape = tuple(alloc.tensor_shape)
            dtype = mybir.dt.np(alloc.dtype)
            out_avals.append(jax.core.ShapedArray(shape, dtype))
            zero_outs.append(np.zeros(shape, dtype))
    n_params = len(in_names)
    all_in = list(in_names) + list(out_names) + ([pname] if pname else [])

    def _body(*args):
        operands = list(args)
        if pname is not None:
            operands.append(bass2jax.partition_id_tensor())
        return tuple(bass2jax._bass_exec_p.bind(
            *operands, out_avals=tuple(out_avals), in_names=tuple(all_in),
            out_names=tuple(out_names), lowering_input_output_aliases=(),
            sim_require_finite=False, sim_require_nnan=False, nc=nc))

    devices = jax.devices()[:N_CORES]
    mesh = Mesh(np.asarray(devices), ("core",))
    spec = NamedSharding(mesh, PartitionSpec("core"))
    nio = n_params + len(out_names)
    fn = jax.jit(shard_map(_body, mesh=mesh,
                           in_specs=(PartitionSpec("core"),) * nio,
                           out_specs=(PartitionSpec("core"),) * len(out_names),
                           check_rep=False), keep_unused=True)
    cat = [np.concatenate([np.asarray(in_maps[c][k]) for c in range(N_CORES)],
                          axis=0) for k in in_names]
    cat += [np.zeros((N_CORES * z.shape[0], *z.shape[1:]), z.dtype)
            for z in zero_outs]
    dev_args = [jax.device_put(a, spec) for a in cat]
    return fn, dev_args, out_names, out_avals


def _time_callable(fn, dev_args, n):
    import time as _time
    import jax
    jax.block_until_ready(fn(*dev_args))
    best = float("inf")
    for _ in range(n):
        t0 = _time.perf_counter()
        jax.block_until_ready(fn(*dev_args))
        best = min(best, _time.perf_counter() - t0)
    return best * 1e9


def benchmark(inputs, n=10):
    """Returns (output, est_hw_ns, raw_wall_ns, floor_wall_ns). The axon
    dispatch round-trip (~80 ms) dominates wall time, so the HW estimate is
    the warm-wall delta vs an empty kernel measured the same way."""
    import concourse.bass as bass
    from concourse.tile import TileContext

    nf = bass.Bass("TRN2", num_devices=N_CORES)
    xf = nf.dram_tensor("x", [1, 128], FP32, kind="ExternalInput")
    yf = nf.dram_tensor("y", [1, 128], FP32, kind="ExternalOutput")
    with TileContext(nf) as tcf:
        with tcf.tile_pool(name="p", bufs=1) as pf:
            tt = pf.tile([1, 128], FP32, name="tt")
            nf.sync.dma_start(out=tt[:], in_=xf[:])
            nf.sync.dma_start(out=yf[:], in_=tt[:])
    split_waits(nf)
    ffn, fargs, _, _ = _make_callable(
        nf, [{"x": np.zeros((1, 128), np.float32)}] * N_CORES)
    floor_ns = _time_callable(ffn, fargs, max(n, 20))

    nc = _get_nc()
    in_maps = [host_inputs(inputs, c) for c in range(N_CORES)]
    fn, dev_args, out_names, out_avals = _make_callable(nc, in_maps)
    wall_ns = _time_callable(fn, dev_args, n)
    outs = fn(*dev_args)
    i = out_names.index("out")
    res = np.asarray(outs[i]).reshape(N_CORES, *out_avals[i].shape)[0]
    return np.asarray(res[0], np.float32), wall_ns - floor_ns, wall_ns, floor_ns
